# revision 2
# baseline (speedup 1.0000x reference)
"""Trainium2 Bass kernel for 1D correlation layer (FlowNet-style), v2.

Problem (hardcoded):
  x_1, x_2: [B=8, C=256, H=96, W=320] fp32
  out[b, d, h, w] = sum_c x_1[b,c,h,w] * x2p[b,c,h,w+d],  d in [0, 41)
  where x2p is x_2 zero-padded by 20 on each side of W.

Sharding: data-parallel over batch B across the 8 NeuronCores (one sample
per core).

v2 changes vs v1:
  - Inputs are cast to bf16 on the host: halves the dominant input-DMA
    traffic (62.9MB -> 31.5MB per core).  bf16 matmul runs at 1 cycle/row
    for any moving width (f32r needed >=256), so the moving stream is
    trimmed to the exact correlation band.
  - Per h-plane, 3 stationary x1 tiles (M=128,128,64) stream only the
    u-window [w0-20, w0+M+20) clipped to [0,W): N = 148/168/84 columns
    (vs 3x256 before): PE work drops ~2.3x.
  - The whole PSUM tile is staged to bf16 SBUF (1 copy/tile, full 128
    partitions) and shipped as G-tiles; the fine shear out[d,w]=G[w,w+d]
    stays on the host (reindex + zero edge triangles only).
"""

import numpy as np

B, C, H, W = 8, 256, 96, 320
MAX_DISP = 20
D = 2 * MAX_DISP + 1  # 41
NCORES = 8

HG = 8                  # h-planes per group
NHG = H // HG           # 12 groups
# stationary tiles: (w0, M, u0, N): x1 cols [w0,w0+M), x2 cols [u0,u0+N)
MTILES = [(0, 128, 0, 148), (128, 128, 108, 168), (256, 64, 236, 84)]

_nc_cache = {}


def _build(reps=1, ablate="full"):
    # ablate: "full" | "in" (input DMA only) | "in+mm" (no copies/out-DMA)
    #         | "nocopy" (copies removed: out-DMA ships stage garbage)
    #         | "noout" (no out-DMA)
    import concourse.bacc as bacc
    import concourse.tile as tile
    import concourse.mybir as mybir
    import contextlib

    nc = bacc.Bacc(
        "TRN2",
        target_bir_lowering=False,
        debug=False,
        enable_asserts=False,
        num_devices=NCORES,
    )
    f32 = mybir.dt.float32
    bf16 = mybir.dt.bfloat16

    x1 = nc.dram_tensor("x_1", (C, H, W), bf16, kind="ExternalInput").ap()
    x2 = nc.dram_tensor("x_2", (C, H, W), bf16, kind="ExternalInput").ap()
    # G-tile scratch, one per stationary tile class; layout [hg, q, hh, c]
    scrs = [
        nc.dram_tensor(f"scr{i}", (NHG, M, HG, N), bf16, kind="ExternalOutput").ap()
        for i, (w0, M, u0, N) in enumerate(MTILES)
    ]

    with tile.TileContext(nc) as tc:
        with tc.tile_pool(name="xin", bufs=2) as xpool, \
             tc.tile_pool(name="stg", bufs=2) as spool, \
             tc.tile_pool(name="ps", bufs=8, space="PSUM") as ppool:
            loop_ctx = tc.For_i(0, reps, 1) if reps > 1 else contextlib.nullcontext()
            with loop_ctx:
                rep = 0
                for hg in range(NHG):
                    x1t = []
                    x2t = []
                    for ck in range(2):
                        t1 = xpool.tile(
                            [128, HG * W], bf16,
                            name=f"x1_{rep}_{hg}_{ck}", tag=f"x1c{ck}",
                        )
                        nc.sync.dma_start(
                            out=t1,
                            in_=x1[ck * 128:(ck + 1) * 128,
                                   hg * HG:(hg + 1) * HG, :],
                        )
                        x1t.append(t1)
                        t2 = xpool.tile(
                            [128, HG * W], bf16,
                            name=f"x2_{rep}_{hg}_{ck}", tag=f"x2c{ck}",
                        )
                        nc.sync.dma_start(
                            out=t2,
                            in_=x2[ck * 128:(ck + 1) * 128,
                                   hg * HG:(hg + 1) * HG, :],
                        )
                        x2t.append(t2)

                    stages = []
                    for i, (w0, M, u0, N) in enumerate(MTILES):
                        st = spool.tile(
                            [M, HG, N], bf16,
                            name=f"st_{rep}_{hg}_{i}", tag=f"st{i}",
                        )
                        stages.append(st)

                    if ablate != "in":
                        for hh in range(HG):
                            for mi, (w0, M, u0, N) in enumerate(MTILES):
                                ps = ppool.tile(
                                    [M, 256], f32,
                                    name=f"ps_{rep}_{hg}_{hh}_{mi}", tag="ps",
                                )
                                for ck in range(2):
                                    nc.tensor.matmul(
                                        ps[:, 0:N],
                                        x1t[ck][:, hh * W + w0:hh * W + w0 + M],
                                        x2t[ck][:, hh * W + u0:hh * W + u0 + N],
                                        start=(ck == 0),
                                        stop=(ck == 1),
                                    )
                                if ablate in ("in+mm", "nocopy"):
                                    continue
                                dst = stages[mi][:, hh, :]
                                src = ps[:, 0:N]
                                if mi == 1:
                                    nc.scalar.copy(dst, src)
                                else:
                                    nc.vector.tensor_copy(dst, src)

                    if ablate in ("full", "nocopy"):
                        for i in range(len(MTILES)):
                            nc.sync.dma_start(out=scrs[i][hg], in_=stages[i])

    nc.compile()
    return nc


def _get_nc(reps=1, ablate="full"):
    key = (reps, ablate)
    if key not in _nc_cache:
        _nc_cache[key] = _build(reps, ablate)
    return _nc_cache[key]


def make_in_maps(x_1, x_2):
    import ml_dtypes
    x_1 = np.asarray(x_1)
    x_2 = np.asarray(x_2)
    return [
        {"x_1": np.ascontiguousarray(x_1[b]).astype(ml_dtypes.bfloat16),
         "x_2": np.ascontiguousarray(x_2[b]).astype(ml_dtypes.bfloat16)}
        for b in range(NCORES)
    ]


def _unshear(res_b, out):
    """res_b: dict with scr0 [12,128,8,148], scr1 [12,128,8,168],
    scr2 [12,64,8,84] (bf16).  out: [D, H, W] float32.
    G-tile i holds G[w0+q, u0+c] = sum_c' x1[c',w]*x2[c',u]; the band entry
    out[d, w] = G[w, w+d-20] lives at c = q+d-20-(u0-w0), which is q+d-20
    for tile0 (pad left by 20) and q+d for tiles 1,2 (pad tile2's tail)."""
    st = np.lib.stride_tricks.as_strided
    out_r = out.reshape(D, NHG, HG, W)
    bufs = []
    p0 = np.empty((NHG, 128, HG, 168), dtype=res_b["scr0"].dtype)
    p0[..., 20:] = res_b["scr0"]
    bufs.append(p0)
    bufs.append(res_b["scr1"])
    p2 = np.empty((NHG, 64, HG, 104), dtype=res_b["scr2"].dtype)
    p2[..., 0:84] = res_b["scr2"]
    bufs.append(p2)
    wlo = 0
    for i, (w0, M, u0, N) in enumerate(((0, 128, 0, 148), (128, 128, 108, 168),
                                        (256, 64, 236, 84))):
        b = bufs[i]
        s = b.strides
        v = st(b, shape=(NHG, M, HG, D), strides=(s[0], s[1] + s[3], s[2], s[3]))
        out_r[:, :, :, w0:w0 + M] = v.transpose(3, 0, 2, 1)
    # zero the out-of-range shift positions (reference zero-pads x_2 in W)
    for w in range(MAX_DISP):
        out[:MAX_DISP - w, :, w] = 0.0
    for w in range(W - MAX_DISP, W):
        out[(W + MAX_DISP - 1) - w + 1:, :, w] = 0.0
    return out


def kernel(x_1, x_2):
    from concourse.bass_utils import run_bass_kernel_spmd

    x_1 = np.asarray(x_1)
    x_2 = np.asarray(x_2)
    assert x_1.shape == (B, C, H, W) and x_2.shape == (B, C, H, W)

    nc = _get_nc(1)
    in_maps = make_in_maps(x_1, x_2)
    res = run_bass_kernel_spmd(nc, in_maps, core_ids=list(range(NCORES)))
    out = np.empty((B, D, H, W), np.float32)
    tmp = np.empty((D, H, W), np.float32)
    for b in range(NCORES):
        _unshear(res.results[b], tmp)
        out[b] = tmp
    return out
